# revision 1
# baseline (speedup 1.0000x reference)
"""GCN graph classifier on 8 Trainium2 NeuronCores (Bass/Tile).

Strategy (graph/data parallel per the sharding hint):
- Nodes are split into 8 contiguous ranges aligned to graph boundaries; each
  core owns the destination side of every edge landing in its range (plus one
  self-loop edge per owned node), pooling and the MLP head for its graphs.
- Message passing is computed as agg = dinv * ((S+I) @ (dinv * h)) via
  per-edge row gathers (dma_gather, 256B rows) and one-hot matmuls that
  scatter-accumulate each 128-edge chunk into its 128-node window in PSUM.
- Two launches: layer 1 (aggregates 2-wide x, emits dinv*relu(conv1) rows),
  host reassembles the full hidden table, layer 2 + mean-pool + MLP head.
- All per-core variation (indices, one-hot selectors, degree data) is input
  data; the compiled program is identical across cores (SPMD).

Self-contained: no imports from the problem directory.
"""
import functools
import time

import numpy as np

import concourse.bass as bass
import concourse.bacc as bacc
import concourse.mybir as mybir
import concourse.tile as tile

N_NODES = 100_000
N_PAD = 100_352            # 128-aligned, 3*32768 + 2048
N_EDGES = 1_200_000
N_GRAPHS = 512
HID = 64
NCORES = 8
P = 128
CHUNK_ROWS = 32_768        # int16-addressable table chunk
NCHUNK = 4                 # ceil(N_PAD / CHUNK_ROWS) -> 3*32768 + 2048
BATCH_SLOTS = 8            # slots per gather batch
GMAX = 96                  # padded graphs per core (real ~64)
SENT = 30_000.0            # one-hot sentinel (never matches iota 0..127)

F32 = mybir.dt.float32
I16 = mybir.dt.int16


# ---------------------------------------------------------------- host prep

def _prep(edge_index: np.ndarray, batch: np.ndarray):
    """All index-side preprocessing (sharding metadata). No float math on
    values here - only integer index manipulation derived from the graph
    structure, plus integer degree counts (rsqrt happens on device)."""
    src = np.asarray(edge_index[0], dtype=np.int64)
    dst = np.asarray(edge_index[1], dtype=np.int64)
    batch = np.asarray(batch, dtype=np.int64)

    deg = np.bincount(dst, minlength=N_NODES) + 1  # int degree (self-loop +1)

    gptr = np.searchsorted(batch, np.arange(N_GRAPHS + 1))
    targets = (np.arange(1, NCORES) * N_NODES) // NCORES
    gsplit = np.searchsorted(gptr, targets)
    g0 = np.concatenate([[0], gsplit, [N_GRAPHS]])
    n0s = gptr[g0]  # node range starts per core (len 9)

    order = np.argsort(dst, kind="stable")
    dst_s = dst[order]
    src_s = src[order]
    e0s = np.searchsorted(dst_s, n0s)

    cores = []
    for c in range(NCORES):
        n0, n1 = int(n0s[c]), int(n0s[c + 1])
        es = src_s[e0s[c]:e0s[c + 1]]
        ed = dst_s[e0s[c]:e0s[c + 1]]
        own = np.arange(n0, n1, dtype=np.int64)
        es = np.concatenate([es, own])
        ed = np.concatenate([ed, own])
        slot = (ed - n0) >> 7
        chunk = es >> 15
        o2 = np.lexsort((slot, chunk, slot >> 3))  # (batch, chunk, slot)
        cores.append({
            "n0": n0, "n1": n1, "g0": int(g0[c]), "g1": int(g0[c + 1]),
            "es": es[o2], "ed": ed[o2], "slot": slot[o2], "chunk": chunk[o2],
            "W": int(-(-(n1 - n0) // P)),
        })

    W_SLOTS = max(cr["W"] for cr in cores)
    W_SLOTS = -(-W_SLOTS // BATCH_SLOTS) * BATCH_SLOTS  # pad to batch multiple
    NB = W_SLOTS // BATCH_SLOTS

    # per (slot, chunk) 128-block counts, cross-core max
    counts = np.zeros((NCORES, W_SLOTS, NCHUNK), dtype=np.int64)
    for c, cr in enumerate(cores):
        key = cr["slot"] * NCHUNK + cr["chunk"]
        bc = np.bincount(key, minlength=W_SLOTS * NCHUNK)
        counts[c] = bc.reshape(W_SLOTS, NCHUNK)
    K = np.maximum(-(-counts.max(axis=0) // P), 1)  # [W_SLOTS, NCHUNK] blocks

    # block layout: ordered by (batch, chunk, slot-in-batch, k)
    # block_base[s, ch] = index of first block of that piece
    block_base = np.zeros((W_SLOTS, NCHUNK), dtype=np.int64)
    call_meta = []  # per gather call: (chunk, edge_base, n_edges)
    nb_blocks = 0
    for b in range(NB):
        sl = slice(b * BATCH_SLOTS, (b + 1) * BATCH_SLOTS)
        for ch in range(NCHUNK):
            kb = K[sl, ch]
            block_base[sl, ch] = nb_blocks + np.concatenate([[0], np.cumsum(kb)[:-1]])
            ncall = int(kb.sum()) * P
            call_meta.append((ch, nb_blocks * P, ncall))
            nb_blocks += int(kb.sum())
    NSUB = nb_blocks
    NE_LAY = NSUB * P  # total gather slots per layer

    # per-sub (block) metadata: slot position + start/stop flags, slot-major
    sub_of = []  # in processing order: (sub_idx, slot, is_first, is_last)
    for b in range(NB):
        for s in range(b * BATCH_SLOTS, (b + 1) * BATCH_SLOTS):
            subs = []
            for ch in range(NCHUNK):
                for k in range(int(K[s, ch])):
                    subs.append(int(block_base[s, ch]) + k)
            for i, sub in enumerate(subs):
                sub_of.append((sub, s, i == 0, i == len(subs) - 1))

    # per-core data arrays
    percore = []
    for c, cr in enumerate(cores):
        es, ed, slot, chunk = cr["es"], cr["ed"], cr["slot"], cr["chunk"]
        key = slot * NCHUNK + chunk
        grp_order = np.lexsort((np.arange(len(es)), key))  # already sorted but safe
        # rank within (slot, chunk) group
        sort_key = key[grp_order]
        ranks = np.arange(len(es)) - np.searchsorted(sort_key, sort_key)
        # position of each edge
        pos = block_base[slot[grp_order], chunk[grp_order]] * P + ranks
        esg = es[grp_order]
        edg = ed[grp_order]

        # pad slots gather garbage rows (masked by the one-hot sentinel);
        # spread them across each chunk's rows to avoid hammering one HBM
        # row with thousands of identical descriptors
        rng_pad = np.random.default_rng(12345)
        idx_flat = rng_pad.integers(0, 2048, size=NE_LAY).astype(np.int16)
        idx_flat[pos] = (esg & (CHUNK_ROWS - 1)).astype(np.int16)

        dst_rel = np.full((P, NSUB), SENT, dtype=np.float32)
        dst_rel[pos % P, pos >> 7] = (edg - cr["n0"] - slot[grp_order] * P).astype(np.float32)

        deg_src = np.ones((P, NSUB), dtype=np.float32)
        deg_src[pos % P, pos >> 7] = deg[esg].astype(np.float32)

        deg_own = np.ones((P, W_SLOTS), dtype=np.float32)
        nown = cr["n1"] - cr["n0"]
        ar = np.arange(nown)
        deg_own[ar % P, ar >> 7] = deg[cr["n0"]:cr["n1"]].astype(np.float32)

        g_rel = np.full((P, W_SLOTS), SENT, dtype=np.float32)
        g_rel[ar % P, ar >> 7] = (batch[cr["n0"]:cr["n1"]] - cr["g0"]).astype(np.float32)

        # pack idx into [128, NE_LAY//16] int16 col-major-16 replicated
        cols = NE_LAY // 16
        arr = np.zeros((16, cols), dtype=np.int16)
        j = np.arange(NE_LAY)
        arr[j % 16, j // 16] = idx_flat
        idx_packed = np.tile(arr, (8, 1))

        percore.append({
            **{k: cr[k] for k in ("n0", "n1", "g0", "g1", "W")},
            "idx_packed": idx_packed, "dst_rel": dst_rel,
            "deg_src": deg_src, "deg_own": deg_own, "g_rel": g_rel,
        })

    struct = {
        "W_SLOTS": W_SLOTS, "NB": NB, "NSUB": NSUB, "NE_LAY": NE_LAY,
        "K": K, "block_base": block_base, "call_meta": call_meta,
        "sub_of": sub_of,
    }
    return struct, percore, deg, gptr, n0s, g0


# ------------------------------------------------------------- bass program

def _build_launch(struct, layer: int, reps: int = 1, timing: bool = False, part: str = 'all'):
    """Build the SPMD Bass program for layer 1 or layer 2(+pool+mlp)."""
    W_SLOTS, NB = struct["W_SLOTS"], struct["NB"]
    NSUB, NE_LAY = struct["NSUB"], struct["NE_LAY"]
    K, block_base = struct["K"], struct["block_base"]
    call_meta, sub_of = struct["call_meta"], struct["sub_of"]

    nc = bacc.Bacc("TRN2")
    table = nc.dram_tensor("table", (N_PAD, HID), F32, kind="ExternalInput")
    idx_in = nc.dram_tensor("idx", (P, NE_LAY // 16), I16, kind="ExternalInput")
    dst_rel_in = nc.dram_tensor("dst_rel", (P, NSUB), F32, kind="ExternalInput")
    deg_own_in = nc.dram_tensor("deg_own", (P, W_SLOTS), F32, kind="ExternalInput")
    iota_in = nc.dram_tensor("iota", (P, P), F32, kind="ExternalInput")
    ident_in = nc.dram_tensor("ident", (P, P), F32, kind="ExternalInput")
    ones_in = nc.dram_tensor("ones1", (1, P), F32, kind="ExternalInput")
    if layer == 1:
        deg_src_in = nc.dram_tensor("deg_src", (P, NSUB), F32, kind="ExternalInput")
        w_in = nc.dram_tensor("W1", (2, HID), F32, kind="ExternalInput")
        b_in = nc.dram_tensor("b1", (1, HID), F32, kind="ExternalInput")
        if timing:
            out_t = nc.dram_tensor("h1s_scratch", (W_SLOTS * P, HID), F32)
            dummy_t = nc.dram_tensor("tdummy0", (1, 4), F32, kind="ExternalOutput")
        else:
            out_t = nc.dram_tensor("h1s", (W_SLOTS * P, HID), F32, kind="ExternalOutput")
    else:
        g_rel_in = nc.dram_tensor("g_rel", (P, W_SLOTS), F32, kind="ExternalInput")
        w_in = nc.dram_tensor("W2", (HID, HID), F32, kind="ExternalInput")
        b_in = nc.dram_tensor("b2", (1, HID), F32, kind="ExternalInput")
        wf1_in = nc.dram_tensor("Wf1", (HID, HID), F32, kind="ExternalInput")
        bf1_in = nc.dram_tensor("bf1", (1, HID), F32, kind="ExternalInput")
        wf2_in = nc.dram_tensor("Wf2", (HID, 4), F32, kind="ExternalInput")
        bf2_in = nc.dram_tensor("bf2", (1, 4), F32, kind="ExternalInput")
        out_t = nc.dram_tensor("out", (GMAX, 4), F32, kind="ExternalOutput")

    # organize subs per slot for slot-major processing
    slot_subs = [[] for _ in range(W_SLOTS)]
    for sub, s, first, last in sub_of:
        slot_subs[s].append(sub)

    # sub -> (call index, block-within-call) for gather tile slicing
    sub_call = np.zeros(NSUB, dtype=np.int64)
    sub_kloc = np.zeros(NSUB, dtype=np.int64)
    for ci, (ch, ebase, ncall) in enumerate(call_meta):
        b0 = ebase // P
        nb = ncall // P
        sub_call[b0:b0 + nb] = ci
        sub_kloc[b0:b0 + nb] = np.arange(nb)

    msg_w = 2 if layer == 1 else HID

    with tile.TileContext(nc) as tc:
        with tc.tile_pool(name="const", bufs=1) as cpool, \
             tc.tile_pool(name="meta", bufs=1) as mpool, \
             tc.tile_pool(name="gat", bufs=2) as gpool, \
             tc.tile_pool(name="work", bufs=3) as wpool, \
             tc.tile_pool(name="oh", bufs=3) as ohpool, \
             tc.tile_pool(name="pacc", bufs=3, space="PSUM") as pacc, \
             tc.tile_pool(name="ptp", bufs=2, space="PSUM") as ptp, \
             tc.tile_pool(name="ppool", bufs=1, space="PSUM") as ppool:

            # ---- load constants / metadata
            iota_t = cpool.tile([P, P], F32)
            nc.sync.dma_start(out=iota_t[:], in_=iota_in[:])
            ident_t = cpool.tile([P, P], F32)
            nc.sync.dma_start(out=ident_t[:], in_=ident_in[:])
            ones_t = cpool.tile([1, P], F32)
            nc.sync.dma_start(out=ones_t[:], in_=ones_in[:])
            w_t = cpool.tile([w_in.shape[0], HID], F32)
            nc.sync.dma_start(out=w_t[:], in_=w_in[:])
            b_t = cpool.tile([1, HID], F32)
            nc.sync.dma_start(out=b_t[:], in_=b_in[:])

            idx_t = mpool.tile([P, NE_LAY // 16], I16)
            nc.sync.dma_start(out=idx_t[:], in_=idx_in[:])
            dst_rel_t = mpool.tile([P, NSUB], F32)
            nc.sync.dma_start(out=dst_rel_t[:], in_=dst_rel_in[:])
            deg_own_t = mpool.tile([P, W_SLOTS], F32)
            nc.sync.dma_start(out=deg_own_t[:], in_=deg_own_in[:])

            # dinv_own = sqrt(1/deg_own)
            dinv_own_t = mpool.tile([P, W_SLOTS], F32)
            nc.vector.reciprocal(out=dinv_own_t[:], in_=deg_own_t[:])
            nc.scalar.sqrt(out=dinv_own_t[:], in_=dinv_own_t[:])

            if layer == 1:
                deg_src_t = mpool.tile([P, NSUB], F32)
                nc.sync.dma_start(out=deg_src_t[:], in_=deg_src_in[:])
                dinv_src_t = mpool.tile([P, NSUB], F32)
                nc.vector.reciprocal(out=dinv_src_t[:], in_=deg_src_t[:])
                nc.scalar.sqrt(out=dinv_src_t[:], in_=dinv_src_t[:])
            else:
                g_rel_t = mpool.tile([P, W_SLOTS], F32)
                nc.sync.dma_start(out=g_rel_t[:], in_=g_rel_in[:])
                wf1_t = cpool.tile([HID, HID], F32)
                nc.sync.dma_start(out=wf1_t[:], in_=wf1_in[:])
                wf2_t = cpool.tile([HID, 4], F32)
                nc.sync.dma_start(out=wf2_t[:], in_=wf2_in[:])
                bf1_t = cpool.tile([1, HID], F32)
                nc.sync.dma_start(out=bf1_t[:], in_=bf1_in[:])
                bf2_t = cpool.tile([1, 4], F32)
                nc.sync.dma_start(out=bf2_t[:], in_=bf2_in[:])
                ones_col = cpool.tile([P, 1], F32)
                nc.vector.memset(ones_col[:], 1.0)

            # bias broadcast [P, HID] via ones-matmul
            bb_ps = ptp.tile([P, HID], F32, space="PSUM", tag="hps")
            nc.tensor.matmul(out=bb_ps[:], lhsT=ones_t[:], rhs=b_t[:],
                             start=True, stop=True)
            b_bcast = cpool.tile([P, HID], F32)
            nc.vector.tensor_copy(out=b_bcast[:], in_=bb_ps[:])
            if layer == 2:
                bb2_ps = ptp.tile([P, HID], F32, space="PSUM", tag="hps")
                nc.tensor.matmul(out=bb2_ps[:], lhsT=ones_t[:], rhs=bf1_t[:],
                                 start=True, stop=True)
                bf1_bcast = cpool.tile([P, HID], F32)
                nc.vector.tensor_copy(out=bf1_bcast[:], in_=bb2_ps[:])
                bb3_ps = ptp.tile([P, 4], F32, space="PSUM", tag="hps")
                nc.tensor.matmul(out=bb3_ps[:], lhsT=ones_t[:], rhs=bf2_t[:],
                                 start=True, stop=True)
                bf2_bcast = cpool.tile([P, 4], F32)
                nc.vector.tensor_copy(out=bf2_bcast[:], in_=bb3_ps[:])
                pool_ps = ppool.tile([GMAX, HID + 1], F32, space="PSUM")

            # ---- main loop over batches (reps>1 repeats the whole
            # edge pass for timing-slope measurement; outputs stay valid
            # because each rep re-derives the same values)
            ncall_per_b = NCHUNK
            for rep, b in [(r, b) for r in range(reps) for b in range(NB)]:
                gtiles = []
                for ci in range(b * ncall_per_b, (b + 1) * ncall_per_b):
                    ch, ebase, ncall = call_meta[ci]
                    g_t = gpool.tile([P, ncall // P, HID], F32, tag=f"g{ci % ncall_per_b}")
                    if part != "compute":
                        nc.gpsimd.dma_gather(
                            out_ap=g_t[:],
                            in_ap=table[ch * CHUNK_ROWS: min((ch + 1) * CHUNK_ROWS, N_PAD), :],
                            idxs_ap=idx_t[:, ebase // 16:(ebase + ncall) // 16],
                            num_idxs=ncall, num_idxs_reg=ncall, elem_size=HID,
                            single_packet=False)
                    else:
                        if b < 2:
                            nc.vector.memset(g_t[:], 0.0)
                    gtiles.append(g_t)

                if part == "gather":
                    continue
                for s in range(b * BATCH_SLOTS, (b + 1) * BATCH_SLOTS):
                    subs = slot_subs[s]
                    acc = pacc.tile([P, msg_w], F32, space="PSUM", tag="acc")
                    for i, sub in enumerate(subs):
                        ci = int(sub_call[sub])
                        kloc = int(sub_kloc[sub])
                        g_t = gtiles[ci % ncall_per_b]
                        oh = ohpool.tile([P, P], F32, tag="oh")
                        if layer == 1:
                            nc.vector.tensor_scalar(
                                out=oh[:], in0=iota_t[:],
                                scalar1=dst_rel_t[:, sub:sub + 1],
                                scalar2=dinv_src_t[:, sub:sub + 1],
                                op0=mybir.AluOpType.is_equal,
                                op1=mybir.AluOpType.mult)
                        else:
                            nc.vector.tensor_scalar(
                                out=oh[:], in0=iota_t[:],
                                scalar1=dst_rel_t[:, sub:sub + 1],
                                scalar2=None,
                                op0=mybir.AluOpType.is_equal)
                        nc.tensor.matmul(
                            out=acc[:], lhsT=oh[:],
                            rhs=g_t[:, kloc, 0:msg_w],
                            start=(i == 0), stop=(i == len(subs) - 1))

                    # ---- slot tail
                    a2 = wpool.tile([P, msg_w], F32, tag="a2")
                    nc.scalar.mul(a2[:], acc[:], dinv_own_t[:, s:s + 1])
                    tp_ps = ptp.tile([msg_w, P], F32, space="PSUM", tag="tp")
                    nc.tensor.transpose(out=tp_ps[:], in_=a2[:], identity=ident_t[:])
                    a2t = wpool.tile([msg_w, P], F32, tag="a2t")
                    nc.vector.tensor_copy(out=a2t[:], in_=tp_ps[:])
                    h_ps = ptp.tile([P, HID], F32, space="PSUM", tag="hps")
                    nc.tensor.matmul(out=h_ps[:], lhsT=a2t[:], rhs=w_t[:],
                                     start=True, stop=True)
                    if layer == 1:
                        y = wpool.tile([P, HID], F32, tag="y")
                        nc.vector.tensor_tensor(out=y[:], in0=h_ps[:], in1=b_bcast[:],
                                                op=mybir.AluOpType.add)
                        h1s = wpool.tile([P, HID], F32, tag="h1s")
                        nc.scalar.activation(
                            out=h1s[:], in_=y[:],
                            func=mybir.ActivationFunctionType.Relu,
                            scale=dinv_own_t[:, s:s + 1])
                        nc.sync.dma_start(out=out_t[s * P:(s + 1) * P, :], in_=h1s[:])
                    else:
                        h2 = wpool.tile([P, HID + 1], F32, tag="h2")
                        nc.vector.tensor_tensor(out=h2[:, 0:HID], in0=h_ps[:],
                                                in1=b_bcast[:],
                                                op=mybir.AluOpType.add)
                        nc.scalar.activation(
                            out=h2[:, 0:HID], in_=h2[:, 0:HID],
                            func=mybir.ActivationFunctionType.Relu)
                        nc.vector.tensor_copy(out=h2[:, HID:HID + 1], in_=ones_col[:])
                        goh = ohpool.tile([P, GMAX], F32, tag="goh")
                        nc.vector.tensor_scalar(
                            out=goh[:], in0=iota_t[:, 0:GMAX],
                            scalar1=g_rel_t[:, s:s + 1], scalar2=None,
                            op0=mybir.AluOpType.is_equal)
                        nc.tensor.matmul(out=pool_ps[:], lhsT=goh[:], rhs=h2[:],
                                         start=(s == 0), stop=(s == W_SLOTS - 1))

            if layer == 1 and timing:
                d = wpool.tile([1, 4], F32, tag="dmy")
                nc.vector.memset(d[:], 0.0)
                nc.sync.dma_start(out=dummy_t[:], in_=d[:])

            # ---- pool + MLP head (layer 2)
            if layer == 2:
                pool_sb = wpool.tile([GMAX, HID + 1], F32, tag="pool")
                nc.vector.tensor_copy(out=pool_sb[:], in_=pool_ps[:])
                cnt = wpool.tile([GMAX, 1], F32, tag="cnt")
                nc.vector.tensor_scalar(
                    out=cnt[:], in0=pool_sb[:, HID:HID + 1], scalar1=1.0,
                    scalar2=None, op0=mybir.AluOpType.max)
                rcnt = wpool.tile([GMAX, 1], F32, tag="rcnt")
                nc.vector.reciprocal(out=rcnt[:], in_=cnt[:])
                means = wpool.tile([GMAX, HID], F32, tag="means")
                nc.scalar.mul(means[:], pool_sb[:, 0:HID], rcnt[:])
                mt_ps = ptp.tile([HID, GMAX], F32, space="PSUM", tag="tp")
                nc.tensor.transpose(out=mt_ps[:], in_=means[:],
                                    identity=ident_t[0:GMAX, 0:GMAX])
                mt = wpool.tile([HID, GMAX], F32, tag="mt")
                nc.vector.tensor_copy(out=mt[:], in_=mt_ps[:])
                f1_ps = ptp.tile([GMAX, HID], F32, space="PSUM", tag="hps")
                nc.tensor.matmul(out=f1_ps[:], lhsT=mt[:], rhs=wf1_t[:],
                                 start=True, stop=True)
                f1 = wpool.tile([GMAX, HID], F32, tag="f1")
                nc.vector.tensor_tensor(out=f1[:], in0=f1_ps[:],
                                        in1=bf1_bcast[0:GMAX, :],
                                        op=mybir.AluOpType.add)
                nc.scalar.activation(out=f1[:], in_=f1[:],
                                     func=mybir.ActivationFunctionType.Relu)
                f1t_ps = ptp.tile([HID, GMAX], F32, space="PSUM", tag="tp")
                nc.tensor.transpose(out=f1t_ps[:], in_=f1[:],
                                    identity=ident_t[0:GMAX, 0:GMAX])
                f1t = wpool.tile([HID, GMAX], F32, tag="f1t")
                nc.vector.tensor_copy(out=f1t[:], in_=f1t_ps[:])
                o_ps = ptp.tile([GMAX, 4], F32, space="PSUM", tag="hps")
                nc.tensor.matmul(out=o_ps[:], lhsT=f1t[:], rhs=wf2_t[:],
                                 start=True, stop=True)
                o_sb = wpool.tile([GMAX, 4], F32, tag="osb")
                nc.vector.tensor_tensor(out=o_sb[:], in0=o_ps[:],
                                        in1=bf2_bcast[0:GMAX, :],
                                        op=mybir.AluOpType.add)
                nc.sync.dma_start(out=out_t[:], in_=o_sb[:])

    nc.finalize()
    return nc


# ---------------------------------------------------------------- pjrt run

class _Runner:
    def __init__(self, nc, n_cores: int = NCORES):
        import jax
        from jax.sharding import Mesh, NamedSharding, PartitionSpec
        from jax.experimental.shard_map import shard_map
        from concourse.bass2jax import (
            _bass_exec_p, install_neuronx_cc_hook, partition_id_tensor)

        install_neuronx_cc_hook()
        self.jax = jax
        self.n_cores = n_cores
        in_names, out_names, out_avals = [], [], []
        pname = nc.partition_id_tensor.name if nc.partition_id_tensor else None
        for alloc in nc.m.functions[0].allocations:
            if not isinstance(alloc, mybir.MemoryLocationSet):
                continue
            name = alloc.memorylocations[0].name
            if alloc.kind == "ExternalInput":
                if name != pname:
                    in_names.append(name)
            elif alloc.kind == "ExternalOutput":
                out_names.append(name)
                out_avals.append(jax.core.ShapedArray(
                    tuple(alloc.tensor_shape), mybir.dt.np(alloc.dtype)))
        self.in_names, self.out_names, self.out_avals = in_names, out_names, out_avals
        n_params, n_outs = len(in_names), len(out_avals)
        all_in = in_names + out_names + ([pname] if pname else [])

        def _body(*args):
            operands = list(args)
            if pname:
                operands.append(partition_id_tensor())
            return tuple(_bass_exec_p.bind(
                *operands, out_avals=tuple(out_avals),
                in_names=tuple(all_in), out_names=tuple(out_names),
                lowering_input_output_aliases=(),
                sim_require_finite=True, sim_require_nnan=True, nc=nc))

        devices = jax.devices()[:n_cores]
        self.mesh = Mesh(np.asarray(devices), ("core",))
        self.sh = NamedSharding(self.mesh, PartitionSpec("core"))
        self.fn = jax.jit(
            shard_map(_body, mesh=self.mesh,
                      in_specs=(PartitionSpec("core"),) * (n_params + n_outs),
                      out_specs=(PartitionSpec("core"),) * n_outs,
                      check_rep=False),
            donate_argnums=tuple(range(n_params, n_params + n_outs)),
            keep_unused=True)
        self._zs = [(n_cores * a.shape[0], *a.shape[1:]) for a in out_avals]
        self._zd = [a.dtype for a in out_avals]
        self._dev_in = None

    def stage(self, in_maps):
        ci = [np.concatenate([np.ascontiguousarray(in_maps[c][n])
                              for c in range(self.n_cores)], axis=0)
              for n in self.in_names]
        self._dev_in = [self.jax.device_put(x, self.sh) for x in ci]
        for x in self._dev_in:
            x.block_until_ready()

    def run(self):
        zeros = [self.jax.device_put(np.zeros(s, d), self.sh)
                 for s, d in zip(self._zs, self._zd)]
        outs = self.fn(*self._dev_in, *zeros)
        for o in outs:
            o.block_until_ready()
        return outs

    def results(self, outs):
        res = []
        for c in range(self.n_cores):
            d = {}
            for i, n in enumerate(self.out_names):
                a = np.asarray(outs[i]).reshape(self.n_cores, *self.out_avals[i].shape)
                d[n] = a[c]
            res.append(d)
        return res


# ----------------------------------------------------------------- kernel()

_CACHE = {}

# timing info from the last kernel() call, for test.py
last_run_info = {}


def _consts():
    iota = np.tile(np.arange(P, dtype=np.float32), (P, 1))
    ident = np.eye(P, dtype=np.float32)
    ones1 = np.ones((1, P), dtype=np.float32)
    return iota, ident, ones1


def kernel(x, edge_index, batch, num_graphs=None, W1=None, b1=None, W2=None,
           b2=None, Wf1=None, bf1=None, Wf2=None, bf2=None):
    x = np.asarray(x, dtype=np.float32)
    W1 = np.asarray(W1, dtype=np.float32)
    b1 = np.asarray(b1, dtype=np.float32).reshape(1, HID)
    W2 = np.asarray(W2, dtype=np.float32)
    b2 = np.asarray(b2, dtype=np.float32).reshape(1, HID)
    Wf1 = np.asarray(Wf1, dtype=np.float32)
    bf1 = np.asarray(bf1, dtype=np.float32).reshape(1, HID)
    Wf2 = np.asarray(Wf2, dtype=np.float32)
    bf2 = np.asarray(bf2, dtype=np.float32).reshape(1, 4)

    ei = np.asarray(edge_index)
    bt = np.asarray(batch)
    key = hash((ei.tobytes(), bt.tobytes()))
    if key not in _CACHE:
        t0 = time.time()
        struct, percore, deg, gptr, n0s, g0 = _prep(ei, bt)
        nc1 = _build_launch(struct, 1)
        r1 = _Runner(nc1)
        nc2 = _build_launch(struct, 2)
        r2 = _Runner(nc2)
        _CACHE[key] = (struct, percore, r1, r2)
        last_run_info["build_s"] = time.time() - t0
    struct, percore, r1, r2 = _CACHE[key]

    iota, ident, ones1 = _consts()

    # launch 1: table = zero-padded x
    xpad = np.zeros((N_PAD, HID), dtype=np.float32)
    xpad[:N_NODES, 0:2] = x

    maps1 = []
    for c in range(NCORES):
        pc = percore[c]
        maps1.append({
            "table": xpad, "idx": pc["idx_packed"], "dst_rel": pc["dst_rel"],
            "deg_src": pc["deg_src"], "deg_own": pc["deg_own"],
            "iota": iota, "ident": ident, "ones1": ones1,
            "W1": W1, "b1": b1,
        })
    t0 = time.time()
    r1.stage(maps1)
    last_run_info["stage1_s"] = time.time() - t0
    t0 = time.time()
    outs1 = r1.run()
    last_run_info["run1_s"] = time.time() - t0
    res1 = r1.results(outs1)

    # host reassembly of the hidden table (pure data movement)
    h1s_full = np.zeros((N_PAD, HID), dtype=np.float32)
    for c in range(NCORES):
        pc = percore[c]
        n0, n1 = pc["n0"], pc["n1"]
        h1s_full[n0:n1] = res1[c]["h1s"][0:n1 - n0]

    maps2 = []
    for c in range(NCORES):
        pc = percore[c]
        maps2.append({
            "table": h1s_full, "idx": pc["idx_packed"], "dst_rel": pc["dst_rel"],
            "deg_own": pc["deg_own"], "g_rel": pc["g_rel"],
            "iota": iota, "ident": ident, "ones1": ones1,
            "W2": W2, "b2": b2, "Wf1": Wf1, "bf1": bf1,
            "Wf2": Wf2, "bf2": bf2,
        })
    t0 = time.time()
    r2.stage(maps2)
    last_run_info["stage2_s"] = time.time() - t0
    t0 = time.time()
    outs2 = r2.run()
    last_run_info["run2_s"] = time.time() - t0
    res2 = r2.results(outs2)

    out = np.zeros((N_GRAPHS, 4), dtype=np.float32)
    for c in range(NCORES):
        pc = percore[c]
        out[pc["g0"]:pc["g1"]] = res2[c]["out"][0:pc["g1"] - pc["g0"]]

    last_run_info["runners"] = (r1, r2)
    last_run_info["maps"] = (maps1, maps2)
    return out


def measure_hw_ns(burst: int = 32):
    """On-device exec time per launch: burst-dispatch a timing variant
    (identical device work; big inter-launch table kept in device scratch)
    and take amortized per-run wall at convergence."""
    import time as _t
    struct, percore, r1, r2 = next(iter(_CACHE.values()))
    maps1, maps2 = last_run_info["maps"]

    detail = {}
    total = 0.0
    for layer, maps in ((1, maps1), (2, maps2)):
        key = ("timing", layer)
        if key not in _CACHE:
            nct = _build_launch(struct, layer, timing=(layer == 1))
            rx = _Runner(nct)
            rx.stage(maps)
            _CACHE[key] = rx
        rx = _CACHE[key]
        rx.run()  # warm
        best = None
        for _ in range(3):
            t0 = _t.perf_counter()
            outs = None
            for _ in range(burst):
                zeros = [rx.jax.device_put(np.zeros(sh, d), rx.sh)
                         for sh, d in zip(rx._zs, rx._zd)]
                outs = rx.fn(*rx._dev_in, *zeros)
            for o in outs:
                o.block_until_ready()
            dt = (_t.perf_counter() - t0) / burst
            best = dt if best is None else min(best, dt)
        detail[f"launch{layer}"] = {"hw_us": round(best * 1e6, 1)}
        total += best
    last_run_info["hw_detail"] = detail
    return total * 1e9

